# revision 7
# baseline (speedup 1.0000x reference)
"""ComplexMultiheadAttention on 8 Trainium2 NeuronCores.

Sharding: core c handles batch b = c//4 and the 4 heads [4*(c%4), 4*(c%4)+4).
Each ComplexLinear is fused into 2 real matmuls with K=2048 over [zr|zi].
The O-projection is row-parallel (Megatron): each core emits a partial
[2048,1024] sum; the host adds the 4 partials per batch plus the exact
bias term (V-bias folds into the output bias because softmax rows sum to 1).

v2: all matmul operands in bf16 (halves DMA/SBUF/LDWEIGHTS, same PE rate),
all 4 heads' QT/KT/V SBUF-resident (no DRAM spill round-trip), softmax
rowsum fully on DVE (bf16 4x mode) instead of PE ones-matmuls, fast
approximate reciprocal, bf16 output partials.

v3 (PE-roofline chasing; trace showed PE busy 87% with 19us dead start,
18us HAM-cold penalty, 28us serialized drain, 11us tail):
- 128 junk warmup matmuls at t=0 keep the PE HAM clock at 2.4GHz so the
  first real matmuls run warm.
- Phase 1 is ft-major (all 8 Q/K PSUM groups accumulate in parallel per
  ft step) with per-ft DMA interleave (z2, wq, wk) so the first matmul
  needs only 3x128KB instead of 4MB.
- O-projection PSUM pool double-buffered (ps_sum merged into ps3) so the
  final drain's matmul groups overlap their activation evacuations.
- Output DMAs split in half to shorten the post-compute tail.
"""

import os
import sys

import numpy as np

sys.path.insert(0, "/opt/trn_rl_repo")

import ml_dtypes

import concourse.bass as bass
import concourse.bacc as bacc_mod
import concourse.mybir as mybir
from concourse.bass_utils import run_bass_kernel_spmd
from concourse.tile import TileContext

try:  # tracing needs antenv.axon_hooks (test harness injects it)
    import antenv.axon_hooks  # noqa: F401
except ImportError:
    os.environ.setdefault("BASS_NEVER_TRACE", "1")

B, L, D, NH = 2, 2048, 1024, 16
HD = D // NH  # 64
N_CORES = 8
NHL = 4  # heads per core
CH2 = 2 * NHL * HD  # 512 local channels, per-head [r(64), i(64)] interleaved
F2 = 2 * D  # 2048 concat feature dim
SCALE = 1.0 / 8.0  # 1/sqrt(HD)

F32 = mybir.dt.float32
BF16 = mybir.dt.bfloat16
AF = mybir.ActivationFunctionType
BF_NP = ml_dtypes.bfloat16


def _build_nc():
    nc = bacc_mod.Bacc(None, target_bir_lowering=False, debug=False)
    z2t = nc.declare_dram_parameter("z2t", [F2, L], BF16, isOutput=False)
    wq = nc.declare_dram_parameter("wq", [F2, CH2], BF16, isOutput=False)
    wk = nc.declare_dram_parameter("wk", [F2, CH2], BF16, isOutput=False)
    wv = nc.declare_dram_parameter("wv", [F2, CH2], BF16, isOutput=False)
    cq = nc.declare_dram_parameter("cq", [CH2], F32, isOutput=False)
    ck = nc.declare_dram_parameter("ck", [CH2], F32, isOutput=False)
    wor = nc.declare_dram_parameter("wor", [CH2, D], BF16, isOutput=False)
    woi = nc.declare_dram_parameter("woi", [CH2, D], BF16, isOutput=False)
    pr = nc.declare_dram_parameter("pr", [L, D], BF16, isOutput=True)
    pi = nc.declare_dram_parameter("pi", [L, D], BF16, isOutput=True)

    NSB = 4
    SBW = L // NSB  # 512 seq cols per block

    with TileContext(nc) as tc:
        with (
            tc.tile_pool(name="persist", bufs=1) as pers,
        ):
            ones_f = pers.tile([128, 1], F32, tag="ones_f")
            nc.vector.memset(ones_f[:], 1.0)
            ones = pers.tile([128, 1], BF16, tag="ones")
            nc.scalar.activation(ones[:], ones_f[:], AF.Copy)
            onesr_f = pers.tile([1, 128], F32, tag="onesr_f")
            nc.vector.memset(onesr_f[:], 1.0)
            warm = pers.tile([128, 8], F32, tag="warm")
            jnk = pers.tile([128, 128], BF16, tag="jnk")
            nc.vector.memset(jnk[:], 0.5)
            cq_sb = pers.tile([128, 4], F32, tag="cq")
            nc.sync.dma_start(cq_sb[:], cq[:].rearrange("(t p) -> p t", p=128))
            ck_sb = pers.tile([128, 4], F32, tag="ck")
            nc.sync.dma_start(ck_sb[:], ck[:].rearrange("(t p) -> p t", p=128))

            # resident Q/K per head [128 ch, L] and V all-heads [128 tok, 16, 512]
            qk_ctx = tc.tile_pool(name="qkv_res", bufs=1)
            qkpool = qk_ctx.__enter__()
            qt_h = [qkpool.tile([128, L], BF16, tag=f"qt_{h}", name=f"qt_{h}")
                    for h in range(NHL)]
            kt_h = [qkpool.tile([128, L], BF16, tag=f"kt_{h}", name=f"kt_{h}")
                    for h in range(NHL)]
            v_all = qkpool.tile([128, 16, CH2], BF16, tag="v_all", name="v_all")

            # ---------- Phase 1: QKV projections, ft-major ----------
            with (
                tc.tile_pool(name="w1", bufs=1) as wpool,
                tc.tile_pool(name="z2", bufs=2) as zpool,
                tc.tile_pool(name="ps1", bufs=1, space="PSUM") as ps1,
            ):
                # PE warmup: ~128 tiny matmuls (~7.5us) flip the HAM clock
                # gate to 8/8 (2.4GHz) while the first DMA waves land, so
                # real matmuls never pay the 1.2GHz cold penalty. They reuse
                # tag psq0 (PSUM is statically allocated per tag; a separate
                # tag would overflow the 8 banks).
                for wi in range(128):
                    wps = ps1.tile([1, 128], F32, tag="psq0", name=f"wps{wi}")
                    nc.tensor.matmul(
                        wps[:], lhsT=ones[:, 0:1], rhs=jnk[:],
                        start=True, stop=True, skip_group_check=True,
                    )

                z2_first = zpool.tile([128, 16, SBW], BF16, tag="z2")
                wq_sb = wpool.tile([128, 16, CH2], BF16, tag="wq")
                wk_sb = wpool.tile([128, 16, CH2], BF16, tag="wk")
                wv_sb = wpool.tile([128, 16, CH2], BF16, tag="wv")

                def z2_ft(tile, sb, ft):
                    nc.sync.dma_start(
                        tile[:, ft, :],
                        z2t[ft * 128 : (ft + 1) * 128,
                            sb * SBW : (sb + 1) * SBW],
                    )

                # first wave interleaved per-ft: the ft=0 matmuls need only
                # z2(ft0)+wq(ft0)+wk(ft0); later fts stream in behind.
                for ft in range(16):
                    z2_ft(z2_first, 0, ft)
                    nc.sync.dma_start(
                        wq_sb[:, ft, :], wq[ft * 128 : (ft + 1) * 128, :]
                    )
                    nc.sync.dma_start(
                        wk_sb[:, ft, :], wk[ft * 128 : (ft + 1) * 128, :]
                    )
                for ft in range(16):
                    nc.sync.dma_start(
                        wv_sb[:, ft, :], wv[ft * 128 : (ft + 1) * 128, :]
                    )

                for sb in range(NSB):
                    if sb == 0:
                        z2_sb = z2_first
                    else:
                        z2_sb = zpool.tile([128, 16, SBW], BF16, tag="z2")
                        for ft in range(16):
                            z2_ft(z2_sb, sb, ft)
                    # Q and K ft-major: 8 PSUM groups accumulate in lockstep
                    # so the first matmul only needs the ft=0 slices.
                    ps_q = [ps1.tile([128, SBW], F32, tag=f"psq{ct}",
                                     name=f"psq{ct}_{sb}")
                            for ct in range(4)]
                    ps_k = [ps1.tile([128, SBW], F32, tag=f"psk{ct}",
                                     name=f"psk{ct}_{sb}")
                            for ct in range(4)]
                    for ft in range(16):
                        for wsb, psl in ((wq_sb, ps_q), (wk_sb, ps_k)):
                            for ct in range(4):
                                nc.tensor.matmul(
                                    psl[ct][:],
                                    lhsT=wsb[:, ft, ct * 128 : (ct + 1) * 128],
                                    rhs=z2_sb[:, ft, :],
                                    start=(ft == 0),
                                    stop=(ft == 15),
                                )
                    for csb, psl, dst in (
                        (cq_sb, ps_q, qt_h),
                        (ck_sb, ps_k, kt_h),
                    ):
                        for ct in range(4):
                            nc.scalar.activation(
                                dst[ct][:, sb * SBW : (sb + 1) * SBW],
                                psl[ct][:],
                                AF.Identity,
                                bias=csb[:, ct : ct + 1],
                            )
                    # V ft-major: 4 PSUM groups (reuse the psq bank tags —
                    # the Q activations have drained them by then)
                    ps_v = [ps1.tile([128, CH2], F32, tag=f"psq{st}",
                                     name=f"psv{st}_{sb}")
                            for st in range(4)]
                    for ft in range(16):
                        for st in range(4):
                            nc.tensor.matmul(
                                ps_v[st][:],
                                lhsT=z2_sb[:, ft, st * 128 : (st + 1) * 128],
                                rhs=wv_sb[:, ft, :],
                                start=(ft == 0),
                                stop=(ft == 15),
                            )
                    for st in range(4):
                        nc.scalar.activation(
                            v_all[:, sb * 4 + st, :], ps_v[st][:], AF.Copy
                        )

            # ---------- Phase 2: attention, flat software pipeline ----------
            # warm up the GpSimd custom-instruction library before phase 2
            # (first partition_broadcast otherwise pays a ~10us IRAM load)
            nc.gpsimd.partition_broadcast(warm[:], onesr_f[0:1, 0:8])
            wo_ctx = tc.tile_pool(name="wo", bufs=1)
            wopool = wo_ctx.__enter__()
            wor_sb = wopool.tile([128, NHL, D], BF16, tag="wor")
            woi_sb = wopool.tile([128, NHL, D], BF16, tag="woi")
            for hh in range(NHL):
                nc.sync.dma_start(wor_sb[:, hh, :], wor[hh * 128 : (hh + 1) * 128, :])
                nc.sync.dma_start(woi_sb[:, hh, :], woi[hh * 128 : (hh + 1) * 128, :])
            # per-qb OT tiles: [128 ch, head, 512 q], per-head [or(64), oi(64)]
            ot_qb = [
                wopool.tile([128, NHL, 512], BF16, tag=f"ot_{qb}", name=f"ot_{qb}")
                for qb in range(4)
            ]

            with (
                tc.tile_pool(name="pstrips", bufs=6) as ppool,
                tc.tile_pool(name="accs", bufs=3) as apool,
                tc.tile_pool(name="small2", bufs=6) as spool,
                tc.tile_pool(name="ev3", bufs=4) as ev3,
                tc.tile_pool(name="ps_s", bufs=2, space="PSUM") as ps_s,
                tc.tile_pool(name="ps_acc", bufs=2, space="PSUM") as ps_acc,
                tc.tile_pool(name="ps3", bufs=2, space="PSUM") as ps3,
            ):
                # qb-major block order; scores+exp processed as strip PAIRS
                # ([128,1024] psum -> one exp ACT) halving scalar-engine time;
                # O-projection groups for finished qb's interleave into the
                # attention pipeline to fill PE bubbles left by the exp chain.
                NAHEAD = 4  # steps (2 pairs) of score lookahead
                NSTEP = NHL * 4 * 16  # 256
                p_tiles = {}

                def step_of(g):
                    return (g // 16) % 4, g // 64, g % 16  # h, qb, kt

                def emit_scores_pair(g):
                    h, qb, kt = step_of(g)
                    sp = ps_s.tile([128, 1024], F32, tag="sp")
                    for j in range(2):
                        nc.tensor.matmul(
                            sp[:, j * 512 : (j + 1) * 512],
                            lhsT=kt_h[h][:, (kt + j) * 128 : (kt + j + 1) * 128],
                            rhs=qt_h[h][:, qb * 512 : (qb + 1) * 512],
                            start=True,
                            stop=True,
                            skip_group_check=True,
                        )
                    p_sb = ppool.tile([128, 1024], BF16, tag="p")
                    nc.scalar.activation(p_sb[:], sp[:], AF.Exp, scale=SCALE)
                    p_tiles[g] = p_sb
                    p_tiles[g + 1] = None  # second half of the pair

                p3_work = []

                def emit_p3_group():
                    if not p3_work:
                        return
                    qb3, qt_local, dst, wsb, nb = p3_work.pop(0)
                    ps = ps3.tile([128, 512], F32, tag="ps3")
                    for h in range(NHL):
                        nc.tensor.matmul(
                            ps[:],
                            lhsT=ot_qb[qb3][
                                :, h, qt_local * 128 : (qt_local + 1) * 128
                            ],
                            rhs=wsb[:, h, nb * 512 : (nb + 1) * 512],
                            start=(h == 0),
                            stop=(h == NHL - 1),
                        )
                    ev = ev3.tile([128, 512], BF16, tag="ev3")
                    nc.scalar.activation(ev[:], ps[:], AF.Copy)
                    qt = qb3 * 4 + qt_local
                    for hf in range(2):
                        nc.sync.dma_start(
                            dst[qt * 128 : (qt + 1) * 128,
                                nb * 512 + hf * 256 : nb * 512 + (hf + 1) * 256],
                            ev[:, hf * 256 : (hf + 1) * 256],
                        )

                pending_tail = None

                def flush_tail():
                    nonlocal pending_tail
                    if pending_tail is None:
                        return
                    av, recip, h, qb = pending_tail
                    rb_sb = spool.tile([128, 512], F32, tag="rb")
                    nc.gpsimd.partition_broadcast(rb_sb[:], recip[:])
                    nc.vector.tensor_mul(ot_qb[qb][:, h, :], av[:], rb_sb[:])
                    pending_tail = None
                    if h == NHL - 1:  # block set for qb complete
                        for qt_local in range(4):
                            for dst, wsb in ((pr, wor_sb), (pi, woi_sb)):
                                for nb in range(2):
                                    p3_work.append((qb, qt_local, dst, wsb, nb))

                for g in range(0, NAHEAD, 2):
                    emit_scores_pair(g)
                av = acc = None
                for g in range(NSTEP):
                    h, qb, kt = step_of(g)
                    if kt == 0:
                        av = ps_acc.tile([128, 512], F32, tag="av")
                        acc = apool.tile([128, 512], BF16, tag="acc")
                    p_sb = p_tiles.pop(g)
                    if p_sb is None:
                        p_sb = p_tiles[-g]  # odd strip: second half of pair
                        p_ap = p_sb[:, 512:1024]
                    else:
                        p_tiles[-(g + 1)] = p_sb
                        p_ap = p_sb[:, 0:512]
                    nc.tensor.matmul(
                        av[:],
                        lhsT=v_all[:, kt, h * 128 : (h + 1) * 128],
                        rhs=p_ap,
                        start=(kt == 0),
                        stop=(kt == 15),
                    )
                    # full rowsum rides DVE (bf16 2x/4x mode); one [1,512]
                    # ones-matmul per block folds it across partitions
                    with nc.allow_low_precision(reason="bf16 strip rowsum"):
                        if kt == 0:
                            nc.vector.tensor_copy(acc[:], p_ap)
                        else:
                            nc.vector.tensor_add(acc[:], acc[:], p_ap)
                    if kt % 2 == 1:
                        p_tiles.pop(-g, None)
                    if g % 2 == 0 and g + NAHEAD < NSTEP:
                        emit_scores_pair(g + NAHEAD)
                    if kt == 2:
                        flush_tail()
                    if kt % 2 == 1:
                        emit_p3_group()
                    if kt == 15:
                        # same tag as the O-proj groups: PSUM tags are
                        # statically allocated, a separate tag would
                        # overflow the 8 banks
                        ssum = ps3.tile([1, 512], F32, tag="ps3",
                                        name=f"ssum{g}")
                        nc.tensor.matmul(
                            ssum[:],
                            lhsT=ones[:, 0:1],
                            rhs=acc[:],
                            start=True,
                            stop=True,
                        )
                        recip = spool.tile([1, 512], F32, tag="recip")
                        nc.vector.reciprocal_approx_fast(recip[:], ssum[:])
                        pending_tail = (av, recip, h, qb)
                flush_tail()
                while p3_work:
                    emit_p3_group()
            wo_ctx.__exit__(None, None, None)
            qk_ctx.__exit__(None, None, None)
    if not nc.is_finalized():
        nc.finalize()
    return nc


_NC = None


def _get_nc():
    global _NC
    if _NC is None:
        _NC = _build_nc()
    return _NC


def _prep(inputs):
    f = lambda k: np.asarray(inputs[k], np.float32)
    zr, zi = f("zr"), f("zi")
    w = {n: f(n) for n in inputs if n not in ("zr", "zi")}

    z2t = [
        np.ascontiguousarray(
            np.concatenate([zr[b].T, zi[b].T], axis=0)
        ).astype(BF_NP)
        for b in range(B)
    ]

    in_maps = []
    for c in range(N_CORES):
        b, hg = c // 4, c % 4
        m = {"z2t": z2t[b]}
        for name in ("q", "k", "v"):
            wr, wi = w[f"w{name}_r"], w[f"w{name}_i"]
            wcat = np.empty((F2, CH2), np.float32)
            for l in range(NHL):
                Ch = np.arange((hg * 4 + l) * HD, (hg * 4 + l + 1) * HD)
                s = l * 128
                wcat[:D, s : s + 64] = wr[Ch, :].T
                wcat[D:, s : s + 64] = -wi[Ch, :].T
                wcat[:D, s + 64 : s + 128] = wi[Ch, :].T
                wcat[D:, s + 64 : s + 128] = wr[Ch, :].T
            m[f"w{name}"] = wcat.astype(BF_NP)
            if name != "v":
                br, bi = w[f"b{name}_r"], w[f"b{name}_i"]
                cb = np.empty((CH2,), np.float32)
                for l in range(NHL):
                    Ch = np.arange((hg * 4 + l) * HD, (hg * 4 + l + 1) * HD)
                    s = l * 128
                    cb[s : s + 64] = br[Ch] - bi[Ch]
                    cb[s + 64 : s + 128] = br[Ch] + bi[Ch]
                m[f"c{name}"] = cb
        wo_r, wo_i = w["wo_r"], w["wo_i"]
        worc = np.empty((CH2, D), np.float32)
        woic = np.empty((CH2, D), np.float32)
        for l in range(NHL):
            Ch = np.arange((hg * 4 + l) * HD, (hg * 4 + l + 1) * HD)
            s = l * 128
            worc[s : s + 64, :] = wo_r[:, Ch].T
            worc[s + 64 : s + 128, :] = -wo_i[:, Ch].T
            woic[s : s + 64, :] = wo_i[:, Ch].T
            woic[s + 64 : s + 128, :] = wo_r[:, Ch].T
        m["wor"] = worc.astype(BF_NP)
        m["woi"] = woic.astype(BF_NP)
        in_maps.append(m)

    # exact host-side bias: V-bias folds through softmax (rows sum to 1)
    cvr = w["bv_r"] - w["bv_i"]
    cvi = w["bv_r"] + w["bv_i"]
    br_total = w["wo_r"] @ cvr - w["wo_i"] @ cvi + w["bo_r"] - w["bo_i"]
    bi_total = w["wo_r"] @ cvi + w["wo_i"] @ cvr + w["bo_r"] + w["bo_i"]
    return in_maps, br_total.astype(np.float32), bi_total.astype(np.float32)


LAST_RESULTS = None


def kernel(**inputs):
    global LAST_RESULTS
    nc = _get_nc()
    in_maps, br_total, bi_total = _prep(inputs)
    res = run_bass_kernel_spmd(nc, in_maps, core_ids=list(range(N_CORES)))
    LAST_RESULTS = res
    out_r = np.zeros((B, L, D), np.float32)
    out_i = np.zeros((B, L, D), np.float32)
    for c in range(N_CORES):
        out_r[c // 4] += np.asarray(res.results[c]["pr"], np.float32)
        out_i[c // 4] += np.asarray(res.results[c]["pi"], np.float32)
    out_r += br_total[None, None, :]
    out_i += bi_total[None, None, :]
    return out_r, out_i


# revision 8
# speedup vs baseline: 1.0563x; 1.0563x over previous
"""ComplexMultiheadAttention on 8 Trainium2 NeuronCores.

Sharding: core c handles batch b = c//4 and the 4 heads [4*(c%4), 4*(c%4)+4).
Each ComplexLinear is fused into 2 real matmuls with K=2048 over [zr|zi].
The O-projection is row-parallel (Megatron): each core emits a partial
[2048,1024] sum; the host adds the 4 partials per batch plus the exact
bias term (V-bias folds into the output bias because softmax rows sum to 1).

v2: all matmul operands in bf16 (halves DMA/SBUF/LDWEIGHTS, same PE rate),
all 4 heads' QT/KT/V SBUF-resident (no DRAM spill round-trip), softmax
rowsum fully on DVE (bf16 4x mode) instead of PE ones-matmuls, fast
approximate reciprocal, bf16 output partials.

v4 (keeps v2's ct-major phase 1 — an ft-major variant that rotated PSUM
banks every matmul hit the PSUM-queue-cycling micro-idle mode and ran the
whole phase cold at 1.2GHz):
- 28 N=512 warmup matmuls at t~9us flip the PE HAM clock gate to 2.4GHz
  during the initial DMA wait, so the first real matmuls run warm.
- O-projection PSUM pool double-buffered (ps_sum merged into ps3's tag)
  so the final drain's matmul groups overlap their evacuations; the
  drain previously serialized AND re-throttled the PE clock.
- O-projection evacuation copies moved from the Scalar engine to DVE:
  phase 2 Scalar time was 193us (128 exp pairs at (1024+352)/1.2ns plus
  64 copies) against 171us of PE work, making ACT the covert critical
  path. DVE sits at ~30% and absorbs the copies.
- Output DMAs split in half to shorten the post-compute tail.
"""

import os
import sys

import numpy as np

sys.path.insert(0, "/opt/trn_rl_repo")

import ml_dtypes

import concourse.bass as bass
import concourse.bacc as bacc_mod
import concourse.mybir as mybir
from concourse.bass_utils import run_bass_kernel_spmd
from concourse.tile import TileContext

try:  # tracing needs antenv.axon_hooks (test harness injects it)
    import antenv.axon_hooks  # noqa: F401
except ImportError:
    os.environ.setdefault("BASS_NEVER_TRACE", "1")

B, L, D, NH = 2, 2048, 1024, 16
HD = D // NH  # 64
N_CORES = 8
NHL = 4  # heads per core
CH2 = 2 * NHL * HD  # 512 local channels, per-head [r(64), i(64)] interleaved
F2 = 2 * D  # 2048 concat feature dim
SCALE = 1.0 / 8.0  # 1/sqrt(HD)

F32 = mybir.dt.float32
BF16 = mybir.dt.bfloat16
AF = mybir.ActivationFunctionType
BF_NP = ml_dtypes.bfloat16


def _build_nc():
    nc = bacc_mod.Bacc(None, target_bir_lowering=False, debug=False)
    z2t = nc.declare_dram_parameter("z2t", [F2, L], BF16, isOutput=False)
    wq = nc.declare_dram_parameter("wq", [F2, CH2], BF16, isOutput=False)
    wk = nc.declare_dram_parameter("wk", [F2, CH2], BF16, isOutput=False)
    wv = nc.declare_dram_parameter("wv", [F2, CH2], BF16, isOutput=False)
    cq = nc.declare_dram_parameter("cq", [CH2], F32, isOutput=False)
    ck = nc.declare_dram_parameter("ck", [CH2], F32, isOutput=False)
    wor = nc.declare_dram_parameter("wor", [CH2, D], BF16, isOutput=False)
    woi = nc.declare_dram_parameter("woi", [CH2, D], BF16, isOutput=False)
    pr = nc.declare_dram_parameter("pr", [L, D], BF16, isOutput=True)
    pi = nc.declare_dram_parameter("pi", [L, D], BF16, isOutput=True)

    NSB = 4
    SBW = L // NSB  # 512 seq cols per block

    with TileContext(nc) as tc:
        with (
            tc.tile_pool(name="persist", bufs=1) as pers,
        ):
            ones_f = pers.tile([128, 1], F32, tag="ones_f")
            nc.vector.memset(ones_f[:], 1.0)
            ones = pers.tile([128, 1], BF16, tag="ones")
            nc.scalar.activation(ones[:], ones_f[:], AF.Copy)
            onesr_f = pers.tile([1, 128], F32, tag="onesr_f")
            nc.vector.memset(onesr_f[:], 1.0)
            warm = pers.tile([128, 8], F32, tag="warm")
            jnk = pers.tile([128, 512], BF16, tag="jnk")
            nc.vector.memset(jnk[:], 0.5)
            cq_sb = pers.tile([128, 4], F32, tag="cq")
            nc.sync.dma_start(cq_sb[:], cq[:].rearrange("(t p) -> p t", p=128))
            ck_sb = pers.tile([128, 4], F32, tag="ck")
            nc.sync.dma_start(ck_sb[:], ck[:].rearrange("(t p) -> p t", p=128))

            # resident Q/K per head [128 ch, L] and V all-heads [128 tok, 16, 512]
            qk_ctx = tc.tile_pool(name="qkv_res", bufs=1)
            qkpool = qk_ctx.__enter__()
            qt_h = [qkpool.tile([128, L], BF16, tag=f"qt_{h}", name=f"qt_{h}")
                    for h in range(NHL)]
            kt_h = [qkpool.tile([128, L], BF16, tag=f"kt_{h}", name=f"kt_{h}")
                    for h in range(NHL)]
            v_all = qkpool.tile([128, 16, CH2], BF16, tag="v_all", name="v_all")

            # ---------- Phase 1: QKV projections ----------
            with (
                tc.tile_pool(name="w1", bufs=1) as wpool,
                tc.tile_pool(name="z2", bufs=2) as zpool,
                tc.tile_pool(name="ps1", bufs=3, space="PSUM") as ps1,
            ):
                # PE warmup: ~28 junk matmuls (~10us worth) flip the HAM
                # clock gate to 8/8 (2.4GHz) during the initial DMA wait so
                # the first real matmuls run warm instead of at 1.2GHz.
                # They cycle the ps1 tag's 3 slots like real groups do.
                for wi in range(28):
                    wps = ps1.tile([1, 512], F32, tag="ps1", name=f"wps{wi}")
                    nc.tensor.matmul(
                        wps[:], lhsT=ones[:, 0:1], rhs=jnk[:],
                        start=True, stop=True, skip_group_check=True,
                    )

                z2_first = zpool.tile([128, 16, SBW], BF16, tag="z2")
                wq_sb = wpool.tile([128, 16, CH2], BF16, tag="wq")
                wk_sb = wpool.tile([128, 16, CH2], BF16, tag="wk")
                wv_sb = wpool.tile([128, 16, CH2], BF16, tag="wv")

                def z2_ft(tile, sb, ft):
                    nc.sync.dma_start(
                        tile[:, ft, :],
                        z2t[ft * 128 : (ft + 1) * 128,
                            sb * SBW : (sb + 1) * SBW],
                    )

                # first wave interleaved per-ft so wq and the sb=0 z2 block
                # finish together (the first PSUM group needs all of both)
                for ft in range(16):
                    z2_ft(z2_first, 0, ft)
                    nc.sync.dma_start(
                        wq_sb[:, ft, :], wq[ft * 128 : (ft + 1) * 128, :]
                    )
                    nc.sync.dma_start(
                        wk_sb[:, ft, :], wk[ft * 128 : (ft + 1) * 128, :]
                    )
                for ft in range(16):
                    nc.sync.dma_start(
                        wv_sb[:, ft, :], wv[ft * 128 : (ft + 1) * 128, :]
                    )

                for sb in range(NSB):
                    if sb == 0:
                        z2_sb = z2_first
                    else:
                        z2_sb = zpool.tile([128, 16, SBW], BF16, tag="z2")
                        for ft in range(16):
                            z2_ft(z2_sb, sb, ft)
                    for wsb, csb, dst in (
                        (wq_sb, cq_sb, qt_h),
                        (wk_sb, ck_sb, kt_h),
                    ):
                        for ct in range(4):
                            ps = ps1.tile([128, SBW], F32, tag="ps1")
                            for ft in range(16):
                                nc.tensor.matmul(
                                    ps[:],
                                    lhsT=wsb[:, ft, ct * 128 : (ct + 1) * 128],
                                    rhs=z2_sb[:, ft, :],
                                    start=(ft == 0),
                                    stop=(ft == 15),
                                )
                            nc.scalar.activation(
                                dst[ct][:, sb * SBW : (sb + 1) * SBW],
                                ps[:],
                                AF.Identity,
                                bias=csb[:, ct : ct + 1],
                            )
                    for st in range(SBW // 128):
                        ps = ps1.tile([128, CH2], F32, tag="psv")
                        for ft in range(16):
                            nc.tensor.matmul(
                                ps[:],
                                lhsT=z2_sb[:, ft, st * 128 : (st + 1) * 128],
                                rhs=wv_sb[:, ft, :],
                                start=(ft == 0),
                                stop=(ft == 15),
                            )
                        ktidx = sb * 4 + st
                        nc.scalar.activation(v_all[:, ktidx, :], ps[:], AF.Copy)

            # ---------- Phase 2: attention, flat software pipeline ----------
            # warm up the GpSimd custom-instruction library before phase 2
            # (first partition_broadcast otherwise pays a ~10us IRAM load)
            nc.gpsimd.partition_broadcast(warm[:], onesr_f[0:1, 0:8])
            wo_ctx = tc.tile_pool(name="wo", bufs=1)
            wopool = wo_ctx.__enter__()
            wor_sb = wopool.tile([128, NHL, D], BF16, tag="wor")
            woi_sb = wopool.tile([128, NHL, D], BF16, tag="woi")
            for hh in range(NHL):
                nc.sync.dma_start(wor_sb[:, hh, :], wor[hh * 128 : (hh + 1) * 128, :])
                nc.sync.dma_start(woi_sb[:, hh, :], woi[hh * 128 : (hh + 1) * 128, :])
            # per-qb OT tiles: [128 ch, head, 512 q], per-head [or(64), oi(64)]
            ot_qb = [
                wopool.tile([128, NHL, 512], BF16, tag=f"ot_{qb}", name=f"ot_{qb}")
                for qb in range(4)
            ]

            with (
                tc.tile_pool(name="pstrips", bufs=6) as ppool,
                tc.tile_pool(name="accs", bufs=3) as apool,
                tc.tile_pool(name="small2", bufs=6) as spool,
                tc.tile_pool(name="ev3", bufs=4) as ev3,
                tc.tile_pool(name="ps_s", bufs=2, space="PSUM") as ps_s,
                tc.tile_pool(name="ps_acc", bufs=2, space="PSUM") as ps_acc,
                tc.tile_pool(name="ps3", bufs=2, space="PSUM") as ps3,
            ):
                # qb-major block order; scores+exp processed as strip PAIRS
                # ([128,1024] psum -> one exp ACT) halving scalar-engine time;
                # O-projection groups for finished qb's interleave into the
                # attention pipeline to fill PE bubbles left by the exp chain.
                NAHEAD = 4  # steps (2 pairs) of score lookahead
                NSTEP = NHL * 4 * 16  # 256
                p_tiles = {}

                def step_of(g):
                    return (g // 16) % 4, g // 64, g % 16  # h, qb, kt

                def emit_scores_pair(g):
                    h, qb, kt = step_of(g)
                    sp = ps_s.tile([128, 1024], F32, tag="sp")
                    for j in range(2):
                        nc.tensor.matmul(
                            sp[:, j * 512 : (j + 1) * 512],
                            lhsT=kt_h[h][:, (kt + j) * 128 : (kt + j + 1) * 128],
                            rhs=qt_h[h][:, qb * 512 : (qb + 1) * 512],
                            start=True,
                            stop=True,
                            skip_group_check=True,
                        )
                    p_sb = ppool.tile([128, 1024], BF16, tag="p")
                    nc.scalar.activation(p_sb[:], sp[:], AF.Exp, scale=SCALE)
                    p_tiles[g] = p_sb
                    p_tiles[g + 1] = None  # second half of the pair

                p3_work = []

                def emit_p3_group():
                    if not p3_work:
                        return
                    qb3, qt_local, dst, wsb, nb = p3_work.pop(0)
                    ps = ps3.tile([128, 512], F32, tag="ps3")
                    for h in range(NHL):
                        nc.tensor.matmul(
                            ps[:],
                            lhsT=ot_qb[qb3][
                                :, h, qt_local * 128 : (qt_local + 1) * 128
                            ],
                            rhs=wsb[:, h, nb * 512 : (nb + 1) * 512],
                            start=(h == 0),
                            stop=(h == NHL - 1),
                        )
                    ev = ev3.tile([128, 512], BF16, tag="ev3")
                    # evacuate on DVE, not ACT: ACT is the phase-2 critical
                    # path (exp chain); DVE has ~70% headroom
                    with nc.allow_low_precision(reason="bf16 psum evac"):
                        nc.vector.tensor_copy(ev[:], ps[:])
                    qt = qb3 * 4 + qt_local
                    for hf in range(2):
                        nc.sync.dma_start(
                            dst[qt * 128 : (qt + 1) * 128,
                                nb * 512 + hf * 256 : nb * 512 + (hf + 1) * 256],
                            ev[:, hf * 256 : (hf + 1) * 256],
                        )

                pending_tail = None

                def flush_tail():
                    nonlocal pending_tail
                    if pending_tail is None:
                        return
                    av, recip, h, qb = pending_tail
                    rb_sb = spool.tile([128, 512], F32, tag="rb")
                    nc.gpsimd.partition_broadcast(rb_sb[:], recip[:])
                    nc.vector.tensor_mul(ot_qb[qb][:, h, :], av[:], rb_sb[:])
                    pending_tail = None
                    if h == NHL - 1:  # block set for qb complete
                        for qt_local in range(4):
                            for dst, wsb in ((pr, wor_sb), (pi, woi_sb)):
                                for nb in range(2):
                                    p3_work.append((qb, qt_local, dst, wsb, nb))

                for g in range(0, NAHEAD, 2):
                    emit_scores_pair(g)
                av = acc = None
                for g in range(NSTEP):
                    h, qb, kt = step_of(g)
                    if kt == 0:
                        av = ps_acc.tile([128, 512], F32, tag="av")
                        acc = apool.tile([128, 512], BF16, tag="acc")
                    p_sb = p_tiles.pop(g)
                    if p_sb is None:
                        p_sb = p_tiles[-g]  # odd strip: second half of pair
                        p_ap = p_sb[:, 512:1024]
                    else:
                        p_tiles[-(g + 1)] = p_sb
                        p_ap = p_sb[:, 0:512]
                    nc.tensor.matmul(
                        av[:],
                        lhsT=v_all[:, kt, h * 128 : (h + 1) * 128],
                        rhs=p_ap,
                        start=(kt == 0),
                        stop=(kt == 15),
                    )
                    # full rowsum rides DVE (bf16 2x/4x mode); one [1,512]
                    # ones-matmul per block folds it across partitions
                    with nc.allow_low_precision(reason="bf16 strip rowsum"):
                        if kt == 0:
                            nc.vector.tensor_copy(acc[:], p_ap)
                        else:
                            nc.vector.tensor_add(acc[:], acc[:], p_ap)
                    if kt % 2 == 1:
                        p_tiles.pop(-g, None)
                    if g % 2 == 0 and g + NAHEAD < NSTEP:
                        emit_scores_pair(g + NAHEAD)
                    if kt == 2:
                        flush_tail()
                    if kt % 2 == 1:
                        emit_p3_group()
                    if kt == 15:
                        # ssum shares the ps3 tag: PSUM tags are statically
                        # allocated and a separate tag would exceed 8 banks
                        ssum = ps3.tile([1, 512], F32, tag="ps3",
                                        name=f"ssum{g}")
                        nc.tensor.matmul(
                            ssum[:],
                            lhsT=ones[:, 0:1],
                            rhs=acc[:],
                            start=True,
                            stop=True,
                        )
                        recip = spool.tile([1, 512], F32, tag="recip")
                        nc.vector.reciprocal_approx_fast(recip[:], ssum[:])
                        pending_tail = (av, recip, h, qb)
                flush_tail()
                while p3_work:
                    emit_p3_group()
            wo_ctx.__exit__(None, None, None)
            qk_ctx.__exit__(None, None, None)
    if not nc.is_finalized():
        nc.finalize()
    return nc


_NC = None


def _get_nc():
    global _NC
    if _NC is None:
        _NC = _build_nc()
    return _NC


def _prep(inputs):
    f = lambda k: np.asarray(inputs[k], np.float32)
    zr, zi = f("zr"), f("zi")
    w = {n: f(n) for n in inputs if n not in ("zr", "zi")}

    z2t = [
        np.ascontiguousarray(
            np.concatenate([zr[b].T, zi[b].T], axis=0)
        ).astype(BF_NP)
        for b in range(B)
    ]

    in_maps = []
    for c in range(N_CORES):
        b, hg = c // 4, c % 4
        m = {"z2t": z2t[b]}
        for name in ("q", "k", "v"):
            wr, wi = w[f"w{name}_r"], w[f"w{name}_i"]
            wcat = np.empty((F2, CH2), np.float32)
            for l in range(NHL):
                Ch = np.arange((hg * 4 + l) * HD, (hg * 4 + l + 1) * HD)
                s = l * 128
                wcat[:D, s : s + 64] = wr[Ch, :].T
                wcat[D:, s : s + 64] = -wi[Ch, :].T
                wcat[:D, s + 64 : s + 128] = wi[Ch, :].T
                wcat[D:, s + 64 : s + 128] = wr[Ch, :].T
            m[f"w{name}"] = wcat.astype(BF_NP)
            if name != "v":
                br, bi = w[f"b{name}_r"], w[f"b{name}_i"]
                cb = np.empty((CH2,), np.float32)
                for l in range(NHL):
                    Ch = np.arange((hg * 4 + l) * HD, (hg * 4 + l + 1) * HD)
                    s = l * 128
                    cb[s : s + 64] = br[Ch] - bi[Ch]
                    cb[s + 64 : s + 128] = br[Ch] + bi[Ch]
                m[f"c{name}"] = cb
        wo_r, wo_i = w["wo_r"], w["wo_i"]
        worc = np.empty((CH2, D), np.float32)
        woic = np.empty((CH2, D), np.float32)
        for l in range(NHL):
            Ch = np.arange((hg * 4 + l) * HD, (hg * 4 + l + 1) * HD)
            s = l * 128
            worc[s : s + 64, :] = wo_r[:, Ch].T
            worc[s + 64 : s + 128, :] = -wo_i[:, Ch].T
            woic[s : s + 64, :] = wo_i[:, Ch].T
            woic[s + 64 : s + 128, :] = wo_r[:, Ch].T
        m["wor"] = worc.astype(BF_NP)
        m["woi"] = woic.astype(BF_NP)
        in_maps.append(m)

    # exact host-side bias: V-bias folds through softmax (rows sum to 1)
    cvr = w["bv_r"] - w["bv_i"]
    cvi = w["bv_r"] + w["bv_i"]
    br_total = w["wo_r"] @ cvr - w["wo_i"] @ cvi + w["bo_r"] - w["bo_i"]
    bi_total = w["wo_r"] @ cvi + w["wo_i"] @ cvr + w["bo_r"] + w["bo_i"]
    return in_maps, br_total.astype(np.float32), bi_total.astype(np.float32)


LAST_RESULTS = None


def kernel(**inputs):
    global LAST_RESULTS
    nc = _get_nc()
    in_maps, br_total, bi_total = _prep(inputs)
    res = run_bass_kernel_spmd(nc, in_maps, core_ids=list(range(N_CORES)))
    LAST_RESULTS = res
    out_r = np.zeros((B, L, D), np.float32)
    out_i = np.zeros((B, L, D), np.float32)
    for c in range(N_CORES):
        out_r[c // 4] += np.asarray(res.results[c]["pr"], np.float32)
        out_i[c // 4] += np.asarray(res.results[c]["pi"], np.float32)
    out_r += br_total[None, None, :]
    out_i += bi_total[None, None, :]
    return out_r, out_i


# revision 12
# speedup vs baseline: 1.1133x; 1.0539x over previous
"""ComplexMultiheadAttention on 8 Trainium2 NeuronCores.

Sharding: core c handles batch b = c//4 and the 4 heads [4*(c%4), 4*(c%4)+4).
Each ComplexLinear is fused into 2 real matmuls with K=2048 over [zr|zi].
The O-projection is row-parallel (Megatron): each core emits a partial
[2048,1024] sum; the host adds the 4 partials per batch plus the exact
bias term (V-bias folds into the output bias because softmax rows sum to 1).

v2: all matmul operands in bf16, QT/KT/V SBUF-resident, softmax rowsum on
DVE, fast approximate reciprocal, bf16 output partials.

v5 (trace-driven on top of v2; v3's ft-major phase 1 and v4's unpipelined
DVE evacuation both regressed and were reverted):
- 36 N=512 warmup matmuls flip the PE HAM clock gate to 2.4GHz during the
  ~19us initial DMA wait, ending just as the first real group's data lands.
- First DMA wave interleaves only z2(sb0) with wq (the two tensors the
  first PSUM group needs); wk/wv follow behind.
- V-projection uses the Gauss 3-multiplication complex trick: per term the
  contraction is 1024 instead of 2048, cutting V-proj matmul work 25%.
  (zr+zi) is built on the idle DVE; the real/imag recombines double as the
  PSUM evacuation. Q/K can't use this (their r|i channels must share one
  128-partition tile for the scores contraction); same for O (attention
  output r/i live on fixed partition ranges).
- O-projection evacuations on DVE but pipelined one slot behind the matmul
  group emission so the in-order DVE never head-of-line blocks the rowsum
  chain (Scalar was the covert phase-2 critical path at ~99% busy: 128 exp
  pairs at ~1.1us each; the copies had to leave, but v4 showed they must
  not sit between rowsum adds while their source group is still on the PE).
- ssum ones-matmuls allocate from the score-pair PSUM tag so the p3 tag is
  a clean 2-slot pipeline; ev3 pool deepened to 8 so the drain is not
  gated by output-DMA latency (v4's drain ran at 1.9us/group on a 4-deep
  pool); output DMAs split in half across queues.
"""

import os
import sys

import numpy as np

sys.path.insert(0, "/opt/trn_rl_repo")

import ml_dtypes

import concourse.bass as bass
import concourse.bacc as bacc_mod
import concourse.mybir as mybir
from concourse.bass_utils import run_bass_kernel_spmd
from concourse.tile import TileContext

try:  # tracing needs antenv.axon_hooks (test harness injects it)
    import antenv.axon_hooks  # noqa: F401
except ImportError:
    os.environ.setdefault("BASS_NEVER_TRACE", "1")

B, L, D, NH = 2, 2048, 1024, 16
HD = D // NH  # 64
N_CORES = 8
NHL = 4  # heads per core
CH2 = 2 * NHL * HD  # 512 local channels, per-head [r(64), i(64)] interleaved
CHC = NHL * HD  # 256 complex channels per core
F2 = 2 * D  # 2048 concat feature dim
SCALE = 1.0 / 8.0  # 1/sqrt(HD)

F32 = mybir.dt.float32
BF16 = mybir.dt.bfloat16
AF = mybir.ActivationFunctionType
BF_NP = ml_dtypes.bfloat16


def _build_nc():
    nc = bacc_mod.Bacc(None, target_bir_lowering=False, debug=False)
    z2t = nc.declare_dram_parameter("z2t", [F2, L], BF16, isOutput=False)
    wq = nc.declare_dram_parameter("wq", [F2, CH2], BF16, isOutput=False)
    wk = nc.declare_dram_parameter("wk", [F2, CH2], BF16, isOutput=False)
    # V projection Gauss terms: [wv_r | wv_i - wv_r | wv_r + wv_i], each
    # [D, 256] (transposed, complex channel order)
    wv3 = nc.declare_dram_parameter("wv3", [3, D, CHC], BF16, isOutput=False)
    cq = nc.declare_dram_parameter("cq", [CH2], F32, isOutput=False)
    ck = nc.declare_dram_parameter("ck", [CH2], F32, isOutput=False)
    wor = nc.declare_dram_parameter("wor", [CH2, D], BF16, isOutput=False)
    woi = nc.declare_dram_parameter("woi", [CH2, D], BF16, isOutput=False)
    pr = nc.declare_dram_parameter("pr", [L, D], BF16, isOutput=True)
    pi = nc.declare_dram_parameter("pi", [L, D], BF16, isOutput=True)

    NSB = 4
    SBW = L // NSB  # 512 seq cols per block

    with TileContext(nc) as tc:
        with (
            tc.tile_pool(name="persist", bufs=1) as pers,
        ):
            ones_f = pers.tile([128, 1], F32, tag="ones_f")
            nc.vector.memset(ones_f[:], 1.0)
            ones = pers.tile([128, 1], BF16, tag="ones")
            nc.scalar.activation(ones[:], ones_f[:], AF.Copy)
            onesr_f = pers.tile([1, 128], F32, tag="onesr_f")
            nc.vector.memset(onesr_f[:], 1.0)
            warm = pers.tile([128, 8], F32, tag="warm")
            jnk = pers.tile([128, 512], BF16, tag="jnk")
            nc.vector.memset(jnk[:], 0.5)
            cq_sb = pers.tile([128, 4], F32, tag="cq")
            nc.sync.dma_start(cq_sb[:], cq[:].rearrange("(t p) -> p t", p=128))
            ck_sb = pers.tile([128, 4], F32, tag="ck")
            nc.sync.dma_start(ck_sb[:], ck[:].rearrange("(t p) -> p t", p=128))

            # resident Q/K per head [128 ch, L] and V all-heads [128 tok, 16, 512]
            qk_ctx = tc.tile_pool(name="qkv_res", bufs=1)
            qkpool = qk_ctx.__enter__()
            qt_h = [qkpool.tile([128, L], BF16, tag=f"qt_{h}", name=f"qt_{h}")
                    for h in range(NHL)]
            kt_h = [qkpool.tile([128, L], BF16, tag=f"kt_{h}", name=f"kt_{h}")
                    for h in range(NHL)]
            v_all = qkpool.tile([128, 16, CH2], BF16, tag="v_all", name="v_all")

            # ---------- Phase 1: QKV projections ----------
            with (
                tc.tile_pool(name="w1", bufs=1) as wpool,
                tc.tile_pool(name="z2", bufs=2) as zpool,
                tc.tile_pool(name="zp", bufs=2) as zppool,
                tc.tile_pool(name="k1c", bufs=2) as k1pool,
                tc.tile_pool(name="ps1", bufs=3, space="PSUM") as ps1,
                tc.tile_pool(name="psv3", bufs=1, space="PSUM") as psv3,
            ):
                # PE warmup: ~36 junk matmuls (~12us of PE time starting
                # near t=9us) flip the HAM clock gate to 8/8 (2.4GHz)
                # during the initial DMA wait and end just as the first
                # real group's data lands, so real matmuls run warm.
                for wi in range(36):
                    wps = ps1.tile([1, 512], F32, tag="ps1", name=f"wps{wi}")
                    nc.tensor.matmul(
                        wps[:], lhsT=ones[:, 0:1], rhs=jnk[:],
                        start=True, stop=True, skip_group_check=True,
                    )

                z2_first = zpool.tile([128, 16, SBW], BF16, tag="z2")
                wq_sb = wpool.tile([128, 16, CH2], BF16, tag="wq")
                wk_sb = wpool.tile([128, 16, CH2], BF16, tag="wk")
                wv3_sb = wpool.tile([128, 3, 8, CHC], BF16, tag="wv3")

                def z2_ft(tile, sb, ft):
                    nc.sync.dma_start(
                        tile[:, ft, :],
                        z2t[ft * 128 : (ft + 1) * 128,
                            sb * SBW : (sb + 1) * SBW],
                    )

                # first wave: only what the first PSUM group needs (z2 sb0
                # + wq); wk and wv3 queue behind it.
                for ft in range(16):
                    z2_ft(z2_first, 0, ft)
                    nc.sync.dma_start(
                        wq_sb[:, ft, :], wq[ft * 128 : (ft + 1) * 128, :]
                    )
                for ft in range(16):
                    nc.sync.dma_start(
                        wk_sb[:, ft, :], wk[ft * 128 : (ft + 1) * 128, :]
                    )
                for t in range(3):
                    for ft in range(8):
                        nc.sync.dma_start(
                            wv3_sb[:, t, ft, :],
                            wv3[t, ft * 128 : (ft + 1) * 128, :],
                        )

                for sb in range(NSB):
                    if sb == 0:
                        z2_sb = z2_first
                    else:
                        z2_sb = zpool.tile([128, 16, SBW], BF16, tag="z2")
                        for ft in range(16):
                            z2_ft(z2_sb, sb, ft)
                    # zr+zi for the Gauss k1 term, built on the idle DVE
                    zp_sb = zppool.tile([128, 8, SBW], BF16, tag="zp")
                    for ft in range(8):
                        with nc.allow_low_precision(reason="bf16 zr+zi"):
                            nc.vector.tensor_add(
                                zp_sb[:, ft, :], z2_sb[:, ft, :],
                                z2_sb[:, ft + 8, :],
                            )
                    for wsb, csb, dst in (
                        (wq_sb, cq_sb, qt_h),
                        (wk_sb, ck_sb, kt_h),
                    ):
                        for ct in range(4):
                            ps = ps1.tile([128, SBW], F32, tag="ps1")
                            for ft in range(16):
                                nc.tensor.matmul(
                                    ps[:],
                                    lhsT=wsb[:, ft, ct * 128 : (ct + 1) * 128],
                                    rhs=z2_sb[:, ft, :],
                                    start=(ft == 0),
                                    stop=(ft == 15),
                                )
                            nc.scalar.activation(
                                dst[ct][:, sb * SBW : (sb + 1) * SBW],
                                ps[:],
                                AF.Identity,
                                bias=csb[:, ct : ct + 1],
                            )
                    # V-projection via Gauss: vr = k1-k3, vi = k1+k2 with
                    # k1=(zr+zi)Wr, k2=zr(Wi-Wr), k3=zi(Wr+Wi); 24 matmuls
                    # of K=128,N=256 per token chunk vs 16 of N=512 before.
                    for st in range(SBW // 128):
                        kt3 = []
                        for t in range(3):
                            kps = psv3.tile([128, CHC], F32, tag=f"pv{t}",
                                            name=f"pv{t}_{sb}_{st}")
                            if t == 0:
                                zsrc = [zp_sb[:, ft, st * 128 : (st + 1) * 128]
                                        for ft in range(8)]
                            elif t == 1:
                                zsrc = [z2_sb[:, ft, st * 128 : (st + 1) * 128]
                                        for ft in range(8)]
                            else:
                                zsrc = [z2_sb[:, ft + 8,
                                              st * 128 : (st + 1) * 128]
                                        for ft in range(8)]
                            for ft in range(8):
                                nc.tensor.matmul(
                                    kps[:],
                                    lhsT=zsrc[ft],
                                    rhs=wv3_sb[:, t, ft, :],
                                    start=(ft == 0),
                                    stop=(ft == 7),
                                )
                            kt3.append(kps)
                        ktidx = sb * 4 + st
                        # DVE reads at most one PSUM operand per op: stage
                        # k1 in SBUF, then combine against k2/k3 in PSUM
                        k1c = k1pool.tile([128, CHC], BF16, tag="k1c")
                        with nc.allow_low_precision(reason="bf16 v combine"):
                            nc.vector.tensor_copy(k1c[:], kt3[0][:])
                        vh = v_all[:, ktidx, :].rearrange(
                            "p (h c) -> p h c", h=NHL
                        )
                        k1h = k1c[:].rearrange("p (h c) -> p h c", h=NHL)
                        k2h = kt3[1][:].rearrange("p (h c) -> p h c", h=NHL)
                        k3h = kt3[2][:].rearrange("p (h c) -> p h c", h=NHL)
                        with nc.allow_low_precision(reason="bf16 v combine"):
                            nc.vector.tensor_sub(
                                vh[:, :, 0:HD], k1h[:], k3h[:]
                            )
                            nc.vector.tensor_add(
                                vh[:, :, HD : 2 * HD], k1h[:], k2h[:]
                            )

            # ---------- Phase 2: attention, flat software pipeline ----------
            # warm up the GpSimd custom-instruction library before phase 2
            # (first partition_broadcast otherwise pays a ~10us IRAM load)
            nc.gpsimd.partition_broadcast(warm[:], onesr_f[0:1, 0:8])
            wo_ctx = tc.tile_pool(name="wo", bufs=1)
            wopool = wo_ctx.__enter__()
            wor_sb = wopool.tile([128, NHL, D], BF16, tag="wor")
            woi_sb = wopool.tile([128, NHL, D], BF16, tag="woi")
            for hh in range(NHL):
                nc.sync.dma_start(wor_sb[:, hh, :], wor[hh * 128 : (hh + 1) * 128, :])
                nc.sync.dma_start(woi_sb[:, hh, :], woi[hh * 128 : (hh + 1) * 128, :])
            # per-qb OT tiles: [128 ch, head, 512 q], per-head [or(64), oi(64)]
            ot_qb = [
                wopool.tile([128, NHL, 512], BF16, tag=f"ot_{qb}", name=f"ot_{qb}")
                for qb in range(4)
            ]

            with (
                tc.tile_pool(name="pstrips", bufs=6) as ppool,
                tc.tile_pool(name="accs", bufs=3) as apool,
                tc.tile_pool(name="small2", bufs=6) as spool,
                tc.tile_pool(name="ev3", bufs=8) as ev3,
                tc.tile_pool(name="ps_s", bufs=2, space="PSUM") as ps_s,
                tc.tile_pool(name="ps_acc", bufs=2, space="PSUM") as ps_acc,
                tc.tile_pool(name="ps3", bufs=2, space="PSUM") as ps3,
            ):
                # qb-major block order; scores+exp processed as strip PAIRS
                # ([128,1024] psum -> one exp ACT) halving scalar-engine time;
                # O-projection groups for finished qb's interleave into the
                # attention pipeline to fill PE bubbles left by the exp chain.
                NAHEAD = 4  # steps (2 pairs) of score lookahead
                NSTEP = NHL * 4 * 16  # 256
                p_tiles = {}

                def step_of(g):
                    return (g // 16) % 4, g // 64, g % 16  # h, qb, kt

                def emit_scores_pair(g):
                    h, qb, kt = step_of(g)
                    sp = ps_s.tile([128, 1024], F32, tag="sp")
                    for j in range(2):
                        nc.tensor.matmul(
                            sp[:, j * 512 : (j + 1) * 512],
                            lhsT=kt_h[h][:, (kt + j) * 128 : (kt + j + 1) * 128],
                            rhs=qt_h[h][:, qb * 512 : (qb + 1) * 512],
                            start=True,
                            stop=True,
                            skip_group_check=True,
                        )
                    p_sb = ppool.tile([128, 1024], BF16, tag="p")
                    nc.scalar.activation(p_sb[:], sp[:], AF.Exp, scale=SCALE)
                    p_tiles[g] = p_sb
                    p_tiles[g + 1] = None  # second half of the pair

                p3_work = []
                evac_q = []

                def flush_p3_evac():
                    """Evacuate the oldest finished O-proj PSUM group.
                    Runs one slot behind emission so the in-order DVE never
                    waits on a group still streaming through the PE."""
                    if not evac_q:
                        return
                    ps, dst, qt, nb = evac_q.pop(0)
                    ev = ev3.tile([128, 512], BF16, tag="ev3")
                    with nc.allow_low_precision(reason="bf16 psum evac"):
                        nc.vector.tensor_copy(ev[:], ps[:])
                    for hf in range(2):
                        nc.sync.dma_start(
                            dst[qt * 128 : (qt + 1) * 128,
                                nb * 512 + hf * 256 : nb * 512 + (hf + 1) * 256],
                            ev[:, hf * 256 : (hf + 1) * 256],
                        )

                def emit_p3_group():
                    if not p3_work:
                        return
                    qb3, qt_local, dst, wsb, nb = p3_work.pop(0)
                    ps = ps3.tile([128, 512], F32, tag="ps3")
                    for h in range(NHL):
                        nc.tensor.matmul(
                            ps[:],
                            lhsT=ot_qb[qb3][
                                :, h, qt_local * 128 : (qt_local + 1) * 128
                            ],
                            rhs=wsb[:, h, nb * 512 : (nb + 1) * 512],
                            start=(h == 0),
                            stop=(h == NHL - 1),
                        )
                    evac_q.append((ps, dst, qb3 * 4 + qt_local, nb))

                pending_tail = None

                def flush_tail():
                    nonlocal pending_tail
                    if pending_tail is None:
                        return
                    av, recip, h, qb = pending_tail
                    rb_sb = spool.tile([128, 512], F32, tag="rb")
                    nc.gpsimd.partition_broadcast(rb_sb[:], recip[:])
                    nc.vector.tensor_mul(ot_qb[qb][:, h, :], av[:], rb_sb[:])
                    pending_tail = None
                    if h == NHL - 1:  # block set for qb complete
                        for qt_local in range(4):
                            for dst, wsb in ((pr, wor_sb), (pi, woi_sb)):
                                for nb in range(2):
                                    p3_work.append((qb, qt_local, dst, wsb, nb))

                for g in range(0, NAHEAD, 2):
                    emit_scores_pair(g)
                av = acc = None
                for g in range(NSTEP):
                    h, qb, kt = step_of(g)
                    if kt == 0:
                        av = ps_acc.tile([128, 512], F32, tag="av")
                        acc = apool.tile([128, 512], BF16, tag="acc")
                    p_sb = p_tiles.pop(g)
                    if p_sb is None:
                        p_sb = p_tiles[-g]  # odd strip: second half of pair
                        p_ap = p_sb[:, 512:1024]
                    else:
                        p_tiles[-(g + 1)] = p_sb
                        p_ap = p_sb[:, 0:512]
                    nc.tensor.matmul(
                        av[:],
                        lhsT=v_all[:, kt, h * 128 : (h + 1) * 128],
                        rhs=p_ap,
                        start=(kt == 0),
                        stop=(kt == 15),
                    )
                    # full rowsum rides DVE (bf16 2x/4x mode); one [1,512]
                    # ones-matmul per block folds it across partitions
                    with nc.allow_low_precision(reason="bf16 strip rowsum"):
                        if kt == 0:
                            nc.vector.tensor_copy(acc[:], p_ap)
                        else:
                            nc.vector.tensor_add(acc[:], acc[:], p_ap)
                    if kt % 2 == 1:
                        p_tiles.pop(-g, None)
                    if g % 2 == 0 and g + NAHEAD < NSTEP:
                        emit_scores_pair(g + NAHEAD)
                    if kt == 2:
                        flush_tail()
                    if kt % 2 == 1:
                        flush_p3_evac()
                        emit_p3_group()
                    if kt == 15:
                        # allocated from the score-pair tag: ps3's 2 slots
                        # stay a clean pipeline for the O-proj groups
                        ssum = ps_s.tile([1, 512], F32, tag="sp",
                                         name=f"ssum{g}")
                        nc.tensor.matmul(
                            ssum[:],
                            lhsT=ones[:, 0:1],
                            rhs=acc[:],
                            start=True,
                            stop=True,
                        )
                        recip = spool.tile([1, 512], F32, tag="recip")
                        nc.vector.reciprocal_approx_fast(recip[:], ssum[:])
                        pending_tail = (av, recip, h, qb)
                flush_tail()
                while p3_work:
                    emit_p3_group()
                    flush_p3_evac()
                while evac_q:
                    flush_p3_evac()
            wo_ctx.__exit__(None, None, None)
            qk_ctx.__exit__(None, None, None)
    if not nc.is_finalized():
        nc.finalize()
    return nc


_NC = None


def _get_nc():
    global _NC
    if _NC is None:
        _NC = _build_nc()
    return _NC


def _prep(inputs):
    f = lambda k: np.asarray(inputs[k], np.float32)
    zr, zi = f("zr"), f("zi")
    w = {n: f(n) for n in inputs if n not in ("zr", "zi")}

    z2t = [
        np.ascontiguousarray(
            np.concatenate([zr[b].T, zi[b].T], axis=0)
        ).astype(BF_NP)
        for b in range(B)
    ]

    in_maps = []
    for c in range(N_CORES):
        b, hg = c // 4, c % 4
        m = {"z2t": z2t[b]}
        for name in ("q", "k"):
            wr, wi = w[f"w{name}_r"], w[f"w{name}_i"]
            wcat = np.empty((F2, CH2), np.float32)
            for l in range(NHL):
                Ch = np.arange((hg * 4 + l) * HD, (hg * 4 + l + 1) * HD)
                s = l * 128
                wcat[:D, s : s + 64] = wr[Ch, :].T
                wcat[D:, s : s + 64] = -wi[Ch, :].T
                wcat[:D, s + 64 : s + 128] = wi[Ch, :].T
                wcat[D:, s + 64 : s + 128] = wr[Ch, :].T
            m[f"w{name}"] = wcat.astype(BF_NP)
            br, bi = w[f"b{name}_r"], w[f"b{name}_i"]
            cb = np.empty((CH2,), np.float32)
            for l in range(NHL):
                Ch = np.arange((hg * 4 + l) * HD, (hg * 4 + l + 1) * HD)
                s = l * 128
                cb[s : s + 64] = br[Ch] - bi[Ch]
                cb[s + 64 : s + 128] = br[Ch] + bi[Ch]
            m[f"c{name}"] = cb
        # V Gauss terms, complex-channel order (head-local 64 ch each)
        Chg = np.arange(hg * 4 * HD, (hg * 4 + 4) * HD)
        wvr = w["wv_r"][Chg, :].T  # [D, 256]
        wvi = w["wv_i"][Chg, :].T
        wv3 = np.stack([wvr, wvi - wvr, wvr + wvi], axis=0)
        m["wv3"] = wv3.astype(BF_NP)
        wo_r, wo_i = w["wo_r"], w["wo_i"]
        worc = np.empty((CH2, D), np.float32)
        woic = np.empty((CH2, D), np.float32)
        for l in range(NHL):
            Ch = np.arange((hg * 4 + l) * HD, (hg * 4 + l + 1) * HD)
            s = l * 128
            worc[s : s + 64, :] = wo_r[:, Ch].T
            worc[s + 64 : s + 128, :] = -wo_i[:, Ch].T
            woic[s : s + 64, :] = wo_i[:, Ch].T
            woic[s + 64 : s + 128, :] = wo_r[:, Ch].T
        m["wor"] = worc.astype(BF_NP)
        m["woi"] = woic.astype(BF_NP)
        in_maps.append(m)

    # exact host-side bias: V-bias folds through softmax (rows sum to 1)
    cvr = w["bv_r"] - w["bv_i"]
    cvi = w["bv_r"] + w["bv_i"]
    br_total = w["wo_r"] @ cvr - w["wo_i"] @ cvi + w["bo_r"] - w["bo_i"]
    bi_total = w["wo_r"] @ cvi + w["wo_i"] @ cvr + w["bo_r"] + w["bo_i"]
    return in_maps, br_total.astype(np.float32), bi_total.astype(np.float32)


LAST_RESULTS = None


def kernel(**inputs):
    global LAST_RESULTS
    nc = _get_nc()
    in_maps, br_total, bi_total = _prep(inputs)
    res = run_bass_kernel_spmd(nc, in_maps, core_ids=list(range(N_CORES)))
    LAST_RESULTS = res
    out_r = np.zeros((B, L, D), np.float32)
    out_i = np.zeros((B, L, D), np.float32)
    for c in range(N_CORES):
        out_r[c // 4] += np.asarray(res.results[c]["pr"], np.float32)
        out_i[c // 4] += np.asarray(res.results[c]["pi"], np.float32)
    out_r += br_total[None, None, :]
    out_i += bi_total[None, None, :]
    return out_r, out_i
